# revision 63
# baseline (speedup 1.0000x reference)
import math
import sys
from types import SimpleNamespace

import numpy as np

sys.path.insert(0, "/opt/trn_rl_repo")

from concourse import bacc, bass, mybir, tile

F32 = mybir.dt.float32
BF16 = mybir.dt.bfloat16
I32 = mybir.dt.int32
I16 = mybir.dt.int16

P = 128
WB = 8


def make_cfg(N=50000, E=800000, D=256, H=8, cores=8, split=None):
    HD = D // H
    NPC = N // cores
    NT = math.ceil(NPC / P)
    NTA = math.ceil(N / P)
    ROWU = ((D + 4 * H + 127) // 128) * 128
    if split is None:
        split = NPC * min(cores, 32767 // NPC)
        split = min(split, N)
    assert split % NPC == 0 and split <= 32767 + 1 and N - split <= 32767 + 1
    return SimpleNamespace(
        N=N, E=E, D=D, H=H, HD=HD, cores=cores, NPC=NPC, NT=NT, NTA=NTA,
        ROWU=ROWU, ROWF=ROWU // 2, SPLIT=split,
        NEG=0.2, KDA=8, KDB=6,
    )



def perm_h_inner(D, H):
    HD = D // H
    j = np.arange(D)
    return (j % H) * HD + j // H


def attn_cols(W, a, H):
    D = W.shape[0]
    HD = W.shape[1] // H
    return np.stack(
        [W[:, h * HD:(h + 1) * HD] @ a[h] for h in range(H)], axis=1
    )


def wrap16(flat, reps=8):
    num = len(flat)
    assert num % 16 == 0
    a = np.zeros((16, num // 16), dtype=np.int16)
    a[np.arange(num) % 16, np.arange(num) // 16] = flat
    return np.tile(a, (reps, 1))


def prep_edges(src, dst, cfg):
    c = cfg
    E = len(src)
    KDA, KDB = c.KDA, c.KDB
    order = np.argsort(dst, kind="stable")
    src_s = src[order].astype(np.int64)
    dst_s = dst[order].astype(np.int64)

    core = dst_s // c.NPC
    loc = dst_s - core * c.NPC
    lt = loc // P
    pos = loc - lt * P
    islow = src_s < c.SPLIT
    region = 1 - islow.astype(np.int64)
    KD_of = np.where(region == 0, KDA, KDB)

    g = (core * c.NT + lt) * 2 + region
    key = g * c.N + dst_s
    order2 = np.argsort(key, kind="stable")
    src_s, dst_s, core, lt, pos, islow, region, g, key, KD_of = (
        a[order2] for a in (src_s, dst_s, core, lt, pos, islow, region, g,
                            key, KD_of))

    runstart = np.r_[True, key[1:] != key[:-1]]
    runid = np.cumsum(runstart) - 1
    startidx = np.flatnonzero(runstart)
    r_in_run = np.arange(E) - startidx[runid]

    isdiag = r_in_run < KD_of
    r2 = r_in_run - KD_of
    pairq = np.maximum(r2, 0) // 2
    sub = np.maximum(r2, 0) % 2

    runlen = np.diff(np.r_[startidx, E])
    run_KD = KD_of[startidx]
    runover = np.maximum(runlen - run_KD, 0)
    runpairs = (runover + 1) // 2
    run_g = g[startidx]
    cum = np.cumsum(runpairs) - runpairs
    gfirst = np.r_[True, run_g[1:] != run_g[:-1]]
    g_base = cum[np.maximum.accumulate(np.where(gfirst, np.arange(len(cum)), 0))]
    base_in_g = cum - g_base

    grouppairs = np.zeros(c.cores * c.NT * 2, dtype=np.int64)
    np.add.at(grouppairs, run_g, runpairs)
    mA = grouppairs[0::2].max()
    mB = grouppairs[1::2].max()
    KSA = 2 * int(math.ceil(mA / P)) if mA > 0 else 0
    KSB = 2 * int(math.ceil(mB / P)) if mB > 0 else 0
    KA = KDA + KSA
    KB = KDB + KSB
    K = KA + KB
    KS = KSA + KSB
    KP = KS // 2 + 1

    pairidx = base_in_g[runid] + pairq
    pp = np.where(isdiag, pos, pairidx % P)
    cpair = pairidx // P
    chunk = np.where(
        isdiag,
        np.where(region == 0, r_in_run, KA + r_in_run),
        np.where(region == 0, KDA + 2 * cpair + sub,
                 KA + KDB + 2 * cpair + sub))

    SENT_A = c.SPLIT
    SENT_B = c.N - c.SPLIT
    srcA = np.full((c.cores, c.NT * KA * P), SENT_A, dtype=np.int64)
    srcB = (np.full((c.cores, c.NT * KB * P), SENT_B, dtype=np.int64)
            if KB else None)
    dstl = np.zeros((c.cores, c.NT * KP * P), dtype=np.int64)
    halfc = (np.arange(c.cores) * c.NPC >= c.SPLIT).astype(np.int64)
    tt, pp_ = np.meshgrid(np.arange(c.NT), np.arange(P), indexing="ij")
    iD = (tt * KP * P + pp_).ravel()
    for ci in range(c.cores):
        dstl[ci, iD] = halfc[ci] * (c.NT * P) + (tt * P + pp_).ravel()
    dstpos_sc = np.full((c.cores, P, c.NT * max(KS, 1)), 255.0, dtype=np.float32)

    low = islow
    iA = lt * (KA * P) + (chunk * P + pp)
    srcA[core[low], iA[low]] = src_s[low]
    if KB:
        hi = ~islow
        iB = lt * (KB * P) + ((chunk - KA) * P + pp)
        srcB[core[hi], iB[hi]] = src_s[hi] - c.SPLIT

    half = (core * c.NPC >= c.SPLIT).astype(np.int64)
    sc = ~isdiag
    cks = np.where(region == 0, chunk - KDA, KSA + (chunk - KA - KDB))
    m = sc
    dstpos_sc[core[m], pp[m], (lt * KS + cks)[m]] = pos[m]
    s0 = sc & (sub == 0)
    cp_glob = 1 + np.where(region == 0, cpair, KSA // 2 + cpair)
    iE = lt * (KP * P) + (cp_glob * P + pp)
    dstl[core[s0], iE[s0]] = ((dst_s - core * c.NPC) + half * (c.NT * P))[s0]

    srcA16 = np.stack([wrap16(srcA[ci]) for ci in range(c.cores)])
    srcB16 = (np.stack([wrap16(srcB[ci]) for ci in range(c.cores)])
              if KB else np.zeros((c.cores, P, 0), np.int16))
    dst16 = np.stack([wrap16(dstl[ci]) for ci in range(c.cores)])
    dims = SimpleNamespace(KDA=KDA, KSA=KSA, KDB=KDB, KSB=KSB,
                           KA=KA, KB=KB, K=K, KS=KS, KP=KP)
    return srcA16, srcB16, dst16, dstpos_sc, dims


def prep_all(inputs, cfg):
    c = cfg
    perm = perm_h_inner(c.D, c.H)
    x = np.asarray(inputs["data"], np.float32)
    src = np.asarray(inputs["src"]).astype(np.int64)
    dst = np.asarray(inputs["dst"]).astype(np.int64)

    def rhs_for(W, al, ar, permute_rows):
        W = np.asarray(W, np.float64)
        Wal = attn_cols(W, np.asarray(al, np.float64), c.H)
        War = attn_cols(W, np.asarray(ar, np.float64), c.H)
        Wp = W[:, perm]
        if permute_rows:
            Wp, Wal, War = Wp[perm], Wal[perm], War[perm]
        return to_bf16(np.concatenate([Wp, Wal, War], axis=1))

    rhs1 = rhs_for(inputs["W1"], inputs["al1"], inputs["ar1"], False)
    rhs2 = rhs_for(inputs["W2"], inputs["al2"], inputs["ar2"], True)
    b1 = np.asarray(inputs["b1"], np.float32)[perm].reshape(1, c.D)
    b2 = np.asarray(inputs["b2"], np.float32)[perm].reshape(1, c.D)
    xT = to_bf16(x.T.copy())

    srcA16, srcB16, dst16, dstpos_sc, dims = prep_edges(src, dst, c)

    erbA, erbB = [], []
    for ci in range(c.cores):
        base = ci * c.NPC
        rows = np.arange(c.NT * P)
        ra = np.where(base < c.SPLIT, base + rows, 0)
        ra = np.clip(ra, 0, c.SPLIT - 1)
        rb = np.where(base >= c.SPLIT, base - c.SPLIT + rows, 0)
        rb = np.clip(rb, 0, max(c.N - c.SPLIT - 1, 0))
        erbA.append(wrap16(ra))
        erbB.append(wrap16(rb))

    meta = SimpleNamespace(perm=perm, dims=dims)
    in_maps = []
    for ci in range(c.cores):
        in_maps.append({
            "xT": xT, "rhs1": rhs1, "rhs2": rhs2, "b1row": b1, "b2row": b2,
            "srcA16": srcA16[ci], "srcB16": srcB16[ci], "dst16": dst16[ci],
            "erbA16": erbA[ci], "erbB16": erbB[ci],
            "dstpos": to_bf16(dstpos_sc[ci]),
        })
    return in_maps, meta


def to_bf16(a):
    import ml_dtypes
    return np.asarray(a).astype(ml_dtypes.bfloat16)


def finalize(results, cfg, meta):
    c = cfg
    parts = [results[ci]["out"][: c.NPC] for ci in range(c.cores)]
    out_p = np.concatenate(parts, axis=0)
    out = np.empty_like(out_p)
    out[:, meta.perm] = out_p
    return out



def build_nc(cfg, dims, debug=False):
    c = cfg
    KDA, KSA, KDB, KSB = dims.KDA, dims.KSA, dims.KDB, dims.KSB
    KA, KB, K, KS, KP = dims.KA, dims.KB, dims.K, dims.KS, dims.KP
    NB = c.N - c.SPLIT
    ERR = c.NT * P

    nc = bacc.Bacc("TRN2", target_bir_lowering=False, debug=debug,
                   num_devices=c.cores)

    xT = nc.declare_dram_parameter("xT", [c.D, c.N], BF16, isOutput=False)
    rhs1 = nc.declare_dram_parameter("rhs1", [c.D, c.D + 2 * c.H], BF16, isOutput=False)
    rhs2 = nc.declare_dram_parameter("rhs2", [c.D, c.D + 2 * c.H], BF16, isOutput=False)
    b1row = nc.declare_dram_parameter("b1row", [1, c.D], F32, isOutput=False)
    b2row = nc.declare_dram_parameter("b2row", [1, c.D], F32, isOutput=False)
    srcA16 = nc.declare_dram_parameter("srcA16", [P, c.NT * KA * 8], I16, isOutput=False)
    if KB:
        srcB16 = nc.declare_dram_parameter("srcB16", [P, c.NT * KB * 8], I16, isOutput=False)
    dst16 = nc.declare_dram_parameter("dst16", [P, c.NT * KP * 8], I16, isOutput=False)
    erbA16 = nc.declare_dram_parameter("erbA16", [P, ERR // 16], I16, isOutput=False)
    erbB16 = nc.declare_dram_parameter("erbB16", [P, ERR // 16], I16, isOutput=False)
    dstpos = nc.declare_dram_parameter("dstpos", [P, c.NT * max(KS, 1)], BF16, isOutput=False)
    out_ext = nc.declare_dram_parameter("out", [c.NT * P, c.D], F32, isOutput=True)

    tabA = nc.dram_tensor("tabA", [c.SPLIT + 1, c.ROWF], F32)
    tabB = nc.dram_tensor("tabB", [NB + 1, c.ROWF], F32)
    er_loc = nc.dram_tensor("er_loc", [2 * ERR, 64], F32)
    TSPL = 24
    hT_loc = nc.dram_tensor("hT_loc", [c.D, c.NT * P], BF16)
    hT_ag = nc.dram_tensor("hT_ag", [c.cores * c.D, c.NT * P], BF16,
                           addr_space="Shared" if c.cores > 4 else "Local")

    DH = c.D + c.H
    WCOL = c.D // 2 + c.H

    with tile.TileContext(nc) as tc:
        with (
            tc.tile_pool(name="const", bufs=1) as constp,
            tc.tile_pool(name="lhs", bufs=3) as lhsp,
            tc.tile_pool(name="erb", bufs=2) as erbp,
            tc.tile_pool(name="ostg", bufs=1) as ostgp,
            tc.tile_pool(name="stg", bufs=3) as stgp,
            tc.tile_pool(name="gath", bufs=4) as gathp,
            tc.tile_pool(name="ert", bufs=3) as ertp,
            tc.tile_pool(name="sall", bufs=2) as sallp,
            tc.tile_pool(name="rhsm", bufs=3) as rhsmp,
            tc.tile_pool(name="small", bufs=3) as smallp,
            tc.tile_pool(name="outp", bufs=3) as outp,
            tc.tile_pool(name="hstg", bufs=2) as hstgp,
            tc.tile_pool(name="psA", bufs=4, space="PSUM") as psA,
            tc.tile_pool(name="psB", bufs=3, space="PSUM") as psB,
            tc.tile_pool(name="psT", bufs=1, space="PSUM") as psT,
        ):
            iota = constp.tile([P, P], BF16, tag="iota")
            nc.gpsimd.iota(iota[:], [[1, P]], channel_multiplier=0,
                           allow_small_or_imprecise_dtypes=True)
            iota_rep = constp.tile([P, P, max(KS, 1)], BF16, tag="iota_rep")
            nc.gpsimd.iota(iota_rep[:], [[1, P], [0, max(KS, 1)]],
                           channel_multiplier=0,
                           allow_small_or_imprecise_dtypes=True)
            sent = constp.tile([1, c.ROWF], F32, tag="sent")
            nc.vector.memset(sent[:], 0.0)
            nc.vector.memset(
                sent[:].bitcast(BF16)[0:1, c.D: c.D + c.H], -1e38)
            nc.sync.dma_start(out=tabA[c.SPLIT: c.SPLIT + 1, :], in_=sent[:])
            nc.sync.dma_start(out=tabB[NB: NB + 1, :], in_=sent[:])
            from concourse.masks import make_identity
            ident = constp.tile([P, P], BF16, tag="ident")
            make_identity(nc, ident[:])

            def load_const(name, param, shape, dt):
                t = constp.tile(shape, dt, tag=name, name=name)
                nc.sync.dma_start(out=t[:], in_=param[:, :])
                return t

            srcA_sb = load_const("srcA_sb", srcA16, [P, c.NT * KA * 8], I16)
            srcB_sb = (load_const("srcB_sb", srcB16, [P, c.NT * KB * 8], I16)
                       if KB else None)
            dst_sb = load_const("dst_sb", dst16, [P, c.NT * KP * 8], I16)
            erbA_sb = load_const("erbA_sb", erbA16, [P, ERR // 16], I16)
            erbB_sb = load_const("erbB_sb", erbB16, [P, ERR // 16], I16)
            dstpos_sb = load_const("dstpos_sb", dstpos, [P, c.NT * max(KS, 1)], BF16)

            rhsW = [[constp.tile([P, c.D + 2 * c.H], BF16,
                                 tag=f"rhsW{l}_{kb}", name=f"rhsW{l}_{kb}")
                     for kb in range(2)] for l in range(2)]
            for l, rt in enumerate([rhs1, rhs2]):
                for kb in range(2):
                    nc.sync.dma_start(out=rhsW[l][kb][:],
                                      in_=rt[kb * P: (kb + 1) * P, :])
            b_bc = [constp.tile([P, c.D], F32, tag=f"bbc{l}", name=f"bbc{l}")
                    for l in range(2)]
            for l, bt in enumerate([b1row, b2row]):
                nc.sync.dma_start(out=b_bc[l][:],
                                  in_=bt[0:1, :].to_broadcast([P, c.D]))

            warm = constp.tile([P, 4], F32, tag="warm")
            warmb = warm[:].bitcast(BF16)
            nc.vector.tensor_copy(out=warm[:, 0:1], in_=dstpos_sb[:, 0:1])
            nc.vector.tensor_copy(out=warmb[:, 0:1], in_=iota[:, 0:1])

            def phase_a(layer):
                CS = TSPL * P
                for blk in range(c.cores):
                    base = blk * c.NPC
                    lts = []
                    for kb in range(2):
                        lt_a = lhsp.tile([P, CS], BF16, tag="lhs_a")
                        lt_b = lhsp.tile([P, c.NPC - CS], BF16, tag="lhs_b")
                        if layer == 0:
                            nc.sync.dma_start(
                                out=lt_a[:],
                                in_=xT[kb * P:(kb + 1) * P, base: base + CS])
                            nc.sync.dma_start(
                                out=lt_b[:],
                                in_=xT[kb * P:(kb + 1) * P,
                                       base + CS: base + c.NPC])
                        else:
                            r0 = blk * c.D + kb * P
                            nc.sync.dma_start(
                                out=lt_a[:], in_=hT_ag[r0: r0 + P, 0:CS])
                            nc.sync.dma_start(
                                out=lt_b[:],
                                in_=hT_ag[r0: r0 + P, CS: c.NPC])
                        lts.append((lt_a, lt_b))
                    tab = tabA if base < c.SPLIT else tabB
                    lbase = base if base < c.SPLIT else base - c.SPLIT
                    for b0 in range(0, c.NT, WB):
                        nt_b = min(WB, c.NT - b0)
                        rows = min(c.NPC - b0 * P, WB * P)
                        stg = stgp.tile([P, WB, WCOL], F32, tag="stg")
                        stgb = stg[:].bitcast(BF16)
                        for j in range(nt_b):
                            t = b0 + j
                            m = min(P, c.NPC - t * P)
                            ps = psA.tile([P, c.D + 2 * c.H], F32, tag="psA")
                            for kb in range(2):
                                if t < TSPL:
                                    lsl = lts[kb][0][:, t * P: t * P + m]
                                else:
                                    c0 = t * P - CS
                                    lsl = lts[kb][1][:, c0: c0 + m]
                                nc.tensor.matmul(
                                    out=ps[:m, :], lhsT=lsl,
                                    rhs=rhsW[layer][kb][:],
                                    start=(kb == 0), stop=(kb == 1))
                            if j % 2 == 0:
                                nc.scalar.copy(
                                    out=stgb[:m, j, : c.D + 2 * c.H],
                                    in_=ps[:m, : c.D + 2 * c.H])
                            else:
                                nc.vector.tensor_copy(
                                    out=stgb[:m, j, : c.D + 2 * c.H],
                                    in_=ps[:m, : c.D + 2 * c.H])
                        ft = rows // P
                        rem = rows - ft * P
                        r0 = lbase + b0 * P
                        if ft:
                            nc.sync.dma_start(
                                out=tab[r0: r0 + ft * P, :WCOL]
                                .rearrange("(t p) f -> p t f", p=P),
                                in_=stg[:, :ft, :])
                        if rem:
                            nc.sync.dma_start(
                                out=tab[r0 + ft * P: r0 + ft * P + rem, :WCOL],
                                in_=stg[:rem, ft, :])

            def build_er_loc():
                start = c.D // 2
                eroff = c.H // 2
                NT1 = (c.NT + 3) // 4
                for half, (tab, idx_sb) in enumerate(
                        [(tabA, erbA_sb), (tabB, erbB_sb)]):
                    for t0, t1 in ((0, NT1), (NT1, 2 * NT1),
                                   (2 * NT1, 3 * NT1), (3 * NT1, c.NT)):
                        nt = t1 - t0
                        eb = erbp.tile([P, NT1, 64], F32, tag="erb")
                        nc.gpsimd.dma_gather(
                            out_ap=eb[:, :nt, :],
                            in_ap=tab[:, start: start + 64],
                            idxs_ap=idx_sb[:, t0 * 8: t1 * 8],
                            num_idxs=nt * P, num_idxs_reg=nt * P,
                            elem_size=64, elem_step=c.ROWF,
                            single_packet=nt * P <= 1024)
                        nc.sync.dma_start(
                            out=er_loc[half * ERR + t0 * P:
                                       half * ERR + t1 * P, 0: c.H // 2]
                            .rearrange("(t p) h -> p t h", p=P),
                            in_=eb[:, :nt, eroff: eroff + c.H // 2])

            def phase_b(layer):
                for b0 in range(0, c.NT, WB):
                    nt_b = min(WB, c.NT - b0)
                    if layer == 0:
                        hstg = hstgp.tile([P, 2, WB, P], BF16, tag="hstg")
                    else:
                        ostg = ostgp.tile([P, WB, c.D], F32, tag="ostg")
                    gtiles = {}
                    for t0 in range(b0, b0 + nt_b):
                        gA2 = gathp.tile([P, 1, KA, c.ROWF], F32, tag="gathA")
                        nc.gpsimd.dma_gather(
                            out_ap=gA2[:, 0, :, :],
                            in_ap=tabA[:, :],
                            idxs_ap=srcA_sb[:, t0 * KA * 8:(t0 + 1) * KA * 8],
                            num_idxs=KA * P, num_idxs_reg=KA * P,
                            elem_size=c.ROWF,
                            single_packet=KA * P <= 1024)
                        gB2 = None
                        if KB:
                            gB2 = gathp.tile([P, 1, KB, c.ROWF], F32,
                                             tag="gathB")
                            nc.gpsimd.dma_gather(
                                out_ap=gB2[:, 0, :, :],
                                in_ap=tabB[:, :],
                                idxs_ap=srcB_sb[:, t0 * KB * 8:(t0 + 1) * KB * 8],
                                num_idxs=KB * P, num_idxs_reg=KB * P,
                                elem_size=c.ROWF,
                                single_packet=KB * P <= 1024)
                        ert2 = ertp.tile([P, 1, KP, 64], F32, tag="ert")
                        nc.gpsimd.dma_gather(
                            out_ap=ert2[:, 0, :, :],
                            in_ap=er_loc[:, :],
                            idxs_ap=dst_sb[:, t0 * KP * 8:(t0 + 1) * KP * 8],
                            num_idxs=KP * P, num_idxs_reg=KP * P,
                            elem_size=64, single_packet=KP * P <= 1024)
                        gtiles[t0] = (gA2, gB2, ert2, 0)
                    for j in range(nt_b):
                        t = b0 + j
                        gA2, gB2, ert2, gi = gtiles[t]
                        gA = gA2[:, gi]
                        gB = gB2[:, gi] if gB2 is not None else None
                        ert = ert2[:, gi] if ert2 is not None else None
                        gbA = gA.bitcast(BF16)
                        gbB = gB.bitcast(BF16) if gB is not None else None
                        ea = smallp.tile([P, K, c.H], F32, tag="eadd")
                        erd_t = ert[:, 0:1, 0: c.H // 2].bitcast(BF16)
                        elA = gbA[:, :, c.D: c.D + c.H]
                        elB = gbB[:, :, c.D: c.D + c.H] if gbB is not None else None
                        nc.vector.tensor_tensor(
                            out=ea[:, 0:KDA, :],
                            in0=elA[:, 0:KDA, :],
                            in1=erd_t.to_broadcast([P, KDA, c.H]),
                            op=mybir.AluOpType.add)
                        nc.vector.tensor_tensor(
                            out=ea[:, KA:KA + KDB, :],
                            in0=elB[:, 0:KDB, :],
                            in1=erd_t.to_broadcast([P, KDB, c.H]),
                            op=mybir.AluOpType.add)
                        if KSA:
                            nc.vector.tensor_tensor(
                                out=ea[:, KDA:KA, :]
                                .rearrange("p (k2 s) h -> p k2 s h", s=2),
                                in0=elA[:, KDA:KA, :]
                                .rearrange("p (k2 s) h -> p k2 s h", s=2),
                                in1=ert[:, 1:1 + KSA // 2, 0: c.H // 2]
                                .bitcast(BF16).unsqueeze(2)
                                .to_broadcast([P, KSA // 2, 2, c.H]),
                                op=mybir.AluOpType.add)
                        if KSB:
                            nc.vector.tensor_tensor(
                                out=ea[:, KA + KDB:K, :]
                                .rearrange("p (k2 s) h -> p k2 s h", s=2),
                                in0=elB[:, KDB:KB, :]
                                .rearrange("p (k2 s) h -> p k2 s h", s=2),
                                in1=ert[:, 1 + KSA // 2:KP, 0: c.H // 2]
                                .bitcast(BF16).unsqueeze(2)
                                .to_broadcast([P, KSB // 2, 2, c.H]),
                                op=mybir.AluOpType.add)
                        tmp = smallp.tile([P, K, c.H], F32, tag="lrtmp")
                        nc.vector.tensor_scalar_mul(tmp[:], ea[:], c.NEG)
                        lr = smallp.tile([P, K, c.H], F32, tag="lrout")
                        nc.vector.tensor_tensor(out=lr[:], in0=ea[:], in1=tmp[:],
                                                op=mybir.AluOpType.max)
                        rm = rhsmp.tile([P, K, DH], BF16, tag="rhsm")
                        nc.scalar.activation(
                            out=rm[:, :, c.D: DH], in_=lr[:],
                            func=mybir.ActivationFunctionType.Exp)
                        nc.vector.tensor_tensor(
                            out=rm[:, 0:KA, : c.D].rearrange(
                                "p k (hd h) -> p k hd h", h=c.H),
                            in0=gbA[:, :, : c.D].rearrange(
                                "p k (hd h) -> p k hd h", h=c.H),
                            in1=rm[:, 0:KA, c.D: DH].unsqueeze(2)
                            .to_broadcast([P, KA, c.HD, c.H]),
                            op=mybir.AluOpType.mult)
                        if KB:
                            nc.vector.tensor_tensor(
                                out=rm[:, KA:K, : c.D].rearrange(
                                    "p k (hd h) -> p k hd h", h=c.H),
                                in0=gbB[:, :, : c.D].rearrange(
                                    "p k (hd h) -> p k hd h", h=c.H),
                                in1=rm[:, KA:K, c.D: DH].unsqueeze(2)
                                .to_broadcast([P, KB, c.HD, c.H]),
                                op=mybir.AluOpType.mult)
                        if KS:
                            s_sc = sallp.tile([P, P, KS], BF16, tag="s_sc")
                            nc.vector.tensor_tensor(
                                out=s_sc[:],
                                in0=iota_rep[:],
                                in1=dstpos_sb[:, t * KS:(t + 1) * KS]
                                .unsqueeze(1).to_broadcast([P, P, KS]),
                                op=mybir.AluOpType.is_equal)
                        ps = psB.tile([P, DH], F32, tag="psB")
                        for ck in range(K):
                            if KDA <= ck < KA:
                                lhsT = s_sc[:, :, ck - KDA]
                            elif ck >= KA + KDB:
                                lhsT = s_sc[:, :, KSA + ck - KA - KDB]
                            else:
                                lhsT = ident[:]
                            nc.tensor.matmul(out=ps[:], lhsT=lhsT,
                                             rhs=rm[:, ck, :],
                                             start=(ck == 0), stop=(ck == K - 1))
                        den = smallp.tile([P, c.H], F32, tag="den")
                        nc.vector.tensor_scalar_max(den[:], ps[:, c.D: DH], 1e-30)
                        rcp = smallp.tile([P, c.H], F32, tag="rcp")
                        nc.vector.reciprocal(rcp[:], den[:])
                        o1 = outp.tile([P, c.D], F32, tag="o1")
                        rcpb = rcp[:].unsqueeze(1).to_broadcast([P, c.HD, c.H])
                        ps4 = ps[:, : c.D].rearrange("p (hd h) -> p hd h", h=c.H)
                        o14 = o1[:].rearrange("p (hd h) -> p hd h", h=c.H)
                        nc.vector.tensor_tensor(out=o14, in0=ps4, in1=rcpb,
                                                op=mybir.AluOpType.mult)
                        nc.vector.tensor_tensor(out=o1[:], in0=o1[:],
                                                in1=b_bc[layer][:],
                                                op=mybir.AluOpType.add)
                        if layer == 0:
                            hb = outp.tile([P, c.D], BF16, tag="hb")
                            nc.scalar.activation(
                                out=hb[:], in_=o1[:],
                                func=mybir.ActivationFunctionType.Relu)
                            for kb in range(2):
                                pst = psT.tile([P, P], F32, tag="psT")
                                nc.tensor.matmul(
                                    out=pst[:],
                                    lhsT=hb[:, kb * P: (kb + 1) * P],
                                    rhs=ident[:], start=True, stop=True)
                                nc.scalar.copy(out=hstg[:, kb, j, :],
                                               in_=pst[:])
                        else:
                            nc.scalar.activation(
                                out=ostg[:, j, :], in_=o1[:],
                                func=mybir.ActivationFunctionType.Relu)
                    if layer == 0:
                        for kb in range(2):
                            nc.sync.dma_start(
                                out=hT_loc[kb * P:(kb + 1) * P,
                                           b0 * P: (b0 + nt_b) * P],
                                in_=hstg[:, kb, :nt_b, :])
                    else:
                        nc.sync.dma_start(
                            out=out_ext[b0 * P: (b0 + nt_b) * P, :]
                            .rearrange("(t p) f -> p t f", p=P),
                            in_=ostg[:, :nt_b, :])

            phase_a(0)
            build_er_loc()
            phase_b(0)
            nc.gpsimd.collective_compute(
                "AllGather", mybir.AluOpType.bypass,
                replica_groups=[list(range(c.cores))],
                ins=[hT_loc[:]], outs=[hT_ag[:]])
            phase_a(1)
            build_er_loc()
            phase_b(1)

    nc.compile()
    return nc



def ref_np(inputs, cfg):
    c = cfg
    x = np.asarray(inputs["data"], np.float64)
    src = np.asarray(inputs["src"]).astype(np.int64)
    dst = np.asarray(inputs["dst"]).astype(np.int64)

    def layer(x, W, al, ar, b):
        N = x.shape[0]
        feat = (x @ np.asarray(W, np.float64)).reshape(N, c.H, c.HD)
        el = np.einsum("nhd,hd->nh", feat, np.asarray(al, np.float64))
        er = np.einsum("nhd,hd->nh", feat, np.asarray(ar, np.float64))
        e = el[src] + er[dst]
        e = np.where(e > 0, e, c.NEG * e)
        m = np.full((N, c.H), -np.inf)
        np.maximum.at(m, dst, e)
        a = np.exp(e - m[dst])
        den = np.zeros((N, c.H))
        np.add.at(den, dst, a)
        alpha = a / den[dst]
        msg = feat[src] * alpha[:, :, None]
        out = np.zeros((N, c.H, c.HD))
        np.add.at(out, dst, msg)
        out = out + np.asarray(b, np.float64).reshape(1, c.H, c.HD)
        return np.maximum(out, 0).reshape(N, c.D)

    h = layer(x, inputs["W1"], inputs["al1"], inputs["ar1"], inputs["b1"])
    h = layer(h, inputs["W2"], inputs["al2"], inputs["ar2"], inputs["b2"])
    return h



_BUILD_CACHE = {}


def kernel(**inputs) -> np.ndarray:
    from concourse.bass_utils import run_bass_kernel_spmd

    cfg = make_cfg(N=50000, E=800000, D=256, H=8, cores=8)
    in_maps, meta = prep_all(inputs, cfg)
    dm = meta.dims
    key = (dm.KA, dm.KB, dm.KS)
    if key not in _BUILD_CACHE:
        _BUILD_CACHE[key] = build_nc(cfg, dm)
    nc = _BUILD_CACHE[key]
    res = run_bass_kernel_spmd(nc, in_maps, list(range(cfg.cores)))
    results = [{"out": res.results[ci]["out"]} for ci in range(cfg.cores)]
    out = finalize(results, cfg, meta)
    return np.ascontiguousarray(out.astype(np.float32))


# revision 64
# speedup vs baseline: 1.0004x; 1.0004x over previous
import math
import sys
from types import SimpleNamespace

import numpy as np

sys.path.insert(0, "/opt/trn_rl_repo")

from concourse import bacc, bass, mybir, tile

F32 = mybir.dt.float32
BF16 = mybir.dt.bfloat16
I32 = mybir.dt.int32
I16 = mybir.dt.int16

P = 128
WB = 8


def make_cfg(N=50000, E=800000, D=256, H=8, cores=8, split=None):
    HD = D // H
    NPC = N // cores
    NT = math.ceil(NPC / P)
    NTA = math.ceil(N / P)
    ROWU = ((D + 4 * H + 127) // 128) * 128
    if split is None:
        split = NPC * min(cores, 32767 // NPC)
        split = min(split, N)
    assert split % NPC == 0 and split <= 32767 + 1 and N - split <= 32767 + 1
    return SimpleNamespace(
        N=N, E=E, D=D, H=H, HD=HD, cores=cores, NPC=NPC, NT=NT, NTA=NTA,
        ROWU=ROWU, ROWF=ROWU // 2, SPLIT=split,
        NEG=0.2, KDA=8, KDB=6,
    )



def perm_h_inner(D, H):
    HD = D // H
    j = np.arange(D)
    return (j % H) * HD + j // H


def attn_cols(W, a, H):
    D = W.shape[0]
    HD = W.shape[1] // H
    return np.stack(
        [W[:, h * HD:(h + 1) * HD] @ a[h] for h in range(H)], axis=1
    )


def wrap16(flat, reps=8):
    num = len(flat)
    assert num % 16 == 0
    a = np.zeros((16, num // 16), dtype=np.int16)
    a[np.arange(num) % 16, np.arange(num) // 16] = flat
    return np.tile(a, (reps, 1))


def prep_edges(src, dst, cfg):
    c = cfg
    E = len(src)
    KDA, KDB = c.KDA, c.KDB
    order = np.argsort(dst, kind="stable")
    src_s = src[order].astype(np.int64)
    dst_s = dst[order].astype(np.int64)

    core = dst_s // c.NPC
    loc = dst_s - core * c.NPC
    lt = loc // P
    pos = loc - lt * P
    islow = src_s < c.SPLIT
    region = 1 - islow.astype(np.int64)
    KD_of = np.where(region == 0, KDA, KDB)

    g = (core * c.NT + lt) * 2 + region
    key = g * c.N + dst_s
    order2 = np.argsort(key, kind="stable")
    src_s, dst_s, core, lt, pos, islow, region, g, key, KD_of = (
        a[order2] for a in (src_s, dst_s, core, lt, pos, islow, region, g,
                            key, KD_of))

    runstart = np.r_[True, key[1:] != key[:-1]]
    runid = np.cumsum(runstart) - 1
    startidx = np.flatnonzero(runstart)
    r_in_run = np.arange(E) - startidx[runid]

    isdiag = r_in_run < KD_of
    r2 = r_in_run - KD_of
    pairq = np.maximum(r2, 0) // 2
    sub = np.maximum(r2, 0) % 2

    runlen = np.diff(np.r_[startidx, E])
    run_KD = KD_of[startidx]
    runover = np.maximum(runlen - run_KD, 0)
    runpairs = (runover + 1) // 2
    run_g = g[startidx]
    cum = np.cumsum(runpairs) - runpairs
    gfirst = np.r_[True, run_g[1:] != run_g[:-1]]
    g_base = cum[np.maximum.accumulate(np.where(gfirst, np.arange(len(cum)), 0))]
    base_in_g = cum - g_base

    grouppairs = np.zeros(c.cores * c.NT * 2, dtype=np.int64)
    np.add.at(grouppairs, run_g, runpairs)
    mA = grouppairs[0::2].max()
    mB = grouppairs[1::2].max()
    KSA = 2 * int(math.ceil(mA / P)) if mA > 0 else 0
    KSB = 2 * int(math.ceil(mB / P)) if mB > 0 else 0
    KA = KDA + KSA
    KB = KDB + KSB
    K = KA + KB
    KS = KSA + KSB
    KP = KS // 2 + 1

    pairidx = base_in_g[runid] + pairq
    pp = np.where(isdiag, pos, pairidx % P)
    cpair = pairidx // P
    chunk = np.where(
        isdiag,
        np.where(region == 0, r_in_run, KA + r_in_run),
        np.where(region == 0, KDA + 2 * cpair + sub,
                 KA + KDB + 2 * cpair + sub))

    SENT_A = c.SPLIT
    SENT_B = c.N - c.SPLIT
    srcA = np.full((c.cores, c.NT * KA * P), SENT_A, dtype=np.int64)
    srcB = (np.full((c.cores, c.NT * KB * P), SENT_B, dtype=np.int64)
            if KB else None)
    dstl = np.zeros((c.cores, c.NT * KP * P), dtype=np.int64)
    halfc = (np.arange(c.cores) * c.NPC >= c.SPLIT).astype(np.int64)
    tt, pp_ = np.meshgrid(np.arange(c.NT), np.arange(P), indexing="ij")
    iD = (tt * KP * P + pp_).ravel()
    for ci in range(c.cores):
        dstl[ci, iD] = halfc[ci] * (c.NT * P) + (tt * P + pp_).ravel()
    dstpos_sc = np.full((c.cores, P, c.NT * max(KS, 1)), 255.0, dtype=np.float32)

    low = islow
    iA = lt * (KA * P) + (chunk * P + pp)
    srcA[core[low], iA[low]] = src_s[low]
    if KB:
        hi = ~islow
        iB = lt * (KB * P) + ((chunk - KA) * P + pp)
        srcB[core[hi], iB[hi]] = src_s[hi] - c.SPLIT

    half = (core * c.NPC >= c.SPLIT).astype(np.int64)
    sc = ~isdiag
    cks = np.where(region == 0, chunk - KDA, KSA + (chunk - KA - KDB))
    m = sc
    dstpos_sc[core[m], pp[m], (lt * KS + cks)[m]] = pos[m]
    s0 = sc & (sub == 0)
    cp_glob = 1 + np.where(region == 0, cpair, KSA // 2 + cpair)
    iE = lt * (KP * P) + (cp_glob * P + pp)
    dstl[core[s0], iE[s0]] = ((dst_s - core * c.NPC) + half * (c.NT * P))[s0]

    srcA16 = np.stack([wrap16(srcA[ci]) for ci in range(c.cores)])
    srcB16 = (np.stack([wrap16(srcB[ci]) for ci in range(c.cores)])
              if KB else np.zeros((c.cores, P, 0), np.int16))
    dst16 = np.stack([wrap16(dstl[ci]) for ci in range(c.cores)])
    dims = SimpleNamespace(KDA=KDA, KSA=KSA, KDB=KDB, KSB=KSB,
                           KA=KA, KB=KB, K=K, KS=KS, KP=KP)
    return srcA16, srcB16, dst16, dstpos_sc, dims


def prep_all(inputs, cfg):
    c = cfg
    perm = perm_h_inner(c.D, c.H)
    x = np.asarray(inputs["data"], np.float32)
    src = np.asarray(inputs["src"]).astype(np.int64)
    dst = np.asarray(inputs["dst"]).astype(np.int64)

    def rhs_for(W, al, ar, permute_rows):
        W = np.asarray(W, np.float64)
        Wal = attn_cols(W, np.asarray(al, np.float64), c.H)
        War = attn_cols(W, np.asarray(ar, np.float64), c.H)
        Wp = W[:, perm]
        if permute_rows:
            Wp, Wal, War = Wp[perm], Wal[perm], War[perm]
        return to_bf16(np.concatenate([Wp, Wal, War], axis=1))

    rhs1 = rhs_for(inputs["W1"], inputs["al1"], inputs["ar1"], False)
    rhs2 = rhs_for(inputs["W2"], inputs["al2"], inputs["ar2"], True)
    b1 = np.asarray(inputs["b1"], np.float32)[perm].reshape(1, c.D)
    b2 = np.asarray(inputs["b2"], np.float32)[perm].reshape(1, c.D)
    xT = to_bf16(x.T.copy())

    srcA16, srcB16, dst16, dstpos_sc, dims = prep_edges(src, dst, c)

    erbA, erbB = [], []
    for ci in range(c.cores):
        base = ci * c.NPC
        rows = np.arange(c.NT * P)
        ra = np.where(base < c.SPLIT, base + rows, 0)
        ra = np.clip(ra, 0, c.SPLIT - 1)
        rb = np.where(base >= c.SPLIT, base - c.SPLIT + rows, 0)
        rb = np.clip(rb, 0, max(c.N - c.SPLIT - 1, 0))
        erbA.append(wrap16(ra))
        erbB.append(wrap16(rb))

    meta = SimpleNamespace(perm=perm, dims=dims)
    in_maps = []
    for ci in range(c.cores):
        in_maps.append({
            "xT": xT, "rhs1": rhs1, "rhs2": rhs2, "b1row": b1, "b2row": b2,
            "srcA16": srcA16[ci], "srcB16": srcB16[ci], "dst16": dst16[ci],
            "erbA16": erbA[ci], "erbB16": erbB[ci],
            "dstpos": to_bf16(dstpos_sc[ci]),
        })
    return in_maps, meta


def to_bf16(a):
    import ml_dtypes
    return np.asarray(a).astype(ml_dtypes.bfloat16)


def finalize(results, cfg, meta):
    c = cfg
    parts = [results[ci]["out"][: c.NPC] for ci in range(c.cores)]
    out_p = np.concatenate(parts, axis=0)
    out = np.empty_like(out_p)
    out[:, meta.perm] = out_p
    return out



def build_nc(cfg, dims, debug=False):
    c = cfg
    KDA, KSA, KDB, KSB = dims.KDA, dims.KSA, dims.KDB, dims.KSB
    KA, KB, K, KS, KP = dims.KA, dims.KB, dims.K, dims.KS, dims.KP
    NB = c.N - c.SPLIT
    ERR = c.NT * P

    nc = bacc.Bacc("TRN2", target_bir_lowering=False, debug=debug,
                   num_devices=c.cores)

    xT = nc.declare_dram_parameter("xT", [c.D, c.N], BF16, isOutput=False)
    rhs1 = nc.declare_dram_parameter("rhs1", [c.D, c.D + 2 * c.H], BF16, isOutput=False)
    rhs2 = nc.declare_dram_parameter("rhs2", [c.D, c.D + 2 * c.H], BF16, isOutput=False)
    b1row = nc.declare_dram_parameter("b1row", [1, c.D], F32, isOutput=False)
    b2row = nc.declare_dram_parameter("b2row", [1, c.D], F32, isOutput=False)
    srcA16 = nc.declare_dram_parameter("srcA16", [P, c.NT * KA * 8], I16, isOutput=False)
    if KB:
        srcB16 = nc.declare_dram_parameter("srcB16", [P, c.NT * KB * 8], I16, isOutput=False)
    dst16 = nc.declare_dram_parameter("dst16", [P, c.NT * KP * 8], I16, isOutput=False)
    erbA16 = nc.declare_dram_parameter("erbA16", [P, ERR // 16], I16, isOutput=False)
    erbB16 = nc.declare_dram_parameter("erbB16", [P, ERR // 16], I16, isOutput=False)
    dstpos = nc.declare_dram_parameter("dstpos", [P, c.NT * max(KS, 1)], BF16, isOutput=False)
    out_ext = nc.declare_dram_parameter("out", [c.NT * P, c.D], F32, isOutput=True)

    tabA = nc.dram_tensor("tabA", [c.SPLIT + 1, c.ROWF], F32)
    tabB = nc.dram_tensor("tabB", [NB + 1, c.ROWF], F32)
    er_loc = nc.dram_tensor("er_loc", [2 * ERR, 64], F32)
    TSPL = 24
    hT_loc = nc.dram_tensor("hT_loc", [c.D, c.NT * P], BF16)
    hT_ag = nc.dram_tensor("hT_ag", [c.cores * c.D, c.NT * P], BF16,
                           addr_space="Shared" if c.cores > 4 else "Local")

    DH = c.D + c.H
    WCOL = c.D // 2 + c.H

    with tile.TileContext(nc) as tc:
        with (
            tc.tile_pool(name="const", bufs=1) as constp,
            tc.tile_pool(name="lhs", bufs=3) as lhsp,
            tc.tile_pool(name="erb", bufs=2) as erbp,
            tc.tile_pool(name="ostg", bufs=1) as ostgp,
            tc.tile_pool(name="stg", bufs=3) as stgp,
            tc.tile_pool(name="gath", bufs=4) as gathp,
            tc.tile_pool(name="ert", bufs=3) as ertp,
            tc.tile_pool(name="sall", bufs=2) as sallp,
            tc.tile_pool(name="rhsm", bufs=3) as rhsmp,
            tc.tile_pool(name="small", bufs=3) as smallp,
            tc.tile_pool(name="outp", bufs=3) as outp,
            tc.tile_pool(name="hstg", bufs=2) as hstgp,
            tc.tile_pool(name="psA", bufs=4, space="PSUM") as psA,
            tc.tile_pool(name="psB", bufs=3, space="PSUM") as psB,
            tc.tile_pool(name="psT", bufs=1, space="PSUM") as psT,
        ):
            iota = constp.tile([P, P], BF16, tag="iota")
            nc.gpsimd.iota(iota[:], [[1, P]], channel_multiplier=0,
                           allow_small_or_imprecise_dtypes=True)
            iota_rep = constp.tile([P, P, max(KS, 1)], BF16, tag="iota_rep")
            nc.gpsimd.iota(iota_rep[:], [[1, P], [0, max(KS, 1)]],
                           channel_multiplier=0,
                           allow_small_or_imprecise_dtypes=True)
            sent = constp.tile([1, c.ROWF], F32, tag="sent")
            nc.vector.memset(sent[:], 0.0)
            nc.vector.memset(
                sent[:].bitcast(BF16)[0:1, c.D: c.D + c.H], -1e38)
            nc.sync.dma_start(out=tabA[c.SPLIT: c.SPLIT + 1, :], in_=sent[:])
            nc.sync.dma_start(out=tabB[NB: NB + 1, :], in_=sent[:])
            from concourse.masks import make_identity
            ident = constp.tile([P, P], BF16, tag="ident")
            make_identity(nc, ident[:])

            def load_const(name, param, shape, dt):
                t = constp.tile(shape, dt, tag=name, name=name)
                nc.sync.dma_start(out=t[:], in_=param[:, :])
                return t

            srcA_sb = load_const("srcA_sb", srcA16, [P, c.NT * KA * 8], I16)
            srcB_sb = (load_const("srcB_sb", srcB16, [P, c.NT * KB * 8], I16)
                       if KB else None)
            dst_sb = load_const("dst_sb", dst16, [P, c.NT * KP * 8], I16)
            erbA_sb = load_const("erbA_sb", erbA16, [P, ERR // 16], I16)
            erbB_sb = load_const("erbB_sb", erbB16, [P, ERR // 16], I16)
            dstpos_sb = load_const("dstpos_sb", dstpos, [P, c.NT * max(KS, 1)], BF16)

            rhsW = [[constp.tile([P, c.D + 2 * c.H], BF16,
                                 tag=f"rhsW{l}_{kb}", name=f"rhsW{l}_{kb}")
                     for kb in range(2)] for l in range(2)]
            for l, rt in enumerate([rhs1, rhs2]):
                for kb in range(2):
                    nc.sync.dma_start(out=rhsW[l][kb][:],
                                      in_=rt[kb * P: (kb + 1) * P, :])
            b_bc = [constp.tile([P, c.D], F32, tag=f"bbc{l}", name=f"bbc{l}")
                    for l in range(2)]
            for l, bt in enumerate([b1row, b2row]):
                nc.sync.dma_start(out=b_bc[l][:],
                                  in_=bt[0:1, :].to_broadcast([P, c.D]))

            warm = constp.tile([P, 4], F32, tag="warm")
            warmb = warm[:].bitcast(BF16)
            nc.vector.tensor_copy(out=warm[:, 0:1], in_=dstpos_sb[:, 0:1])
            nc.vector.tensor_copy(out=warmb[:, 0:1], in_=iota[:, 0:1])

            def phase_a(layer):
                CS = TSPL * P
                for blk in range(c.cores):
                    base = blk * c.NPC
                    lts = []
                    for kb in range(2):
                        lt_a = lhsp.tile([P, CS], BF16, tag="lhs_a")
                        lt_b = lhsp.tile([P, c.NPC - CS], BF16, tag="lhs_b")
                        if layer == 0:
                            nc.sync.dma_start(
                                out=lt_a[:],
                                in_=xT[kb * P:(kb + 1) * P, base: base + CS])
                            nc.sync.dma_start(
                                out=lt_b[:],
                                in_=xT[kb * P:(kb + 1) * P,
                                       base + CS: base + c.NPC])
                        else:
                            r0 = blk * c.D + kb * P
                            nc.sync.dma_start(
                                out=lt_a[:], in_=hT_ag[r0: r0 + P, 0:CS])
                            nc.sync.dma_start(
                                out=lt_b[:],
                                in_=hT_ag[r0: r0 + P, CS: c.NPC])
                        lts.append((lt_a, lt_b))
                    tab = tabA if base < c.SPLIT else tabB
                    lbase = base if base < c.SPLIT else base - c.SPLIT
                    for b0 in range(0, c.NT, WB):
                        nt_b = min(WB, c.NT - b0)
                        rows = min(c.NPC - b0 * P, WB * P)
                        stg = stgp.tile([P, WB, WCOL], F32, tag="stg")
                        stgb = stg[:].bitcast(BF16)
                        for j in range(nt_b):
                            t = b0 + j
                            m = min(P, c.NPC - t * P)
                            ps = psA.tile([P, c.D + 2 * c.H], F32, tag="psA")
                            for kb in range(2):
                                if t < TSPL:
                                    lsl = lts[kb][0][:, t * P: t * P + m]
                                else:
                                    c0 = t * P - CS
                                    lsl = lts[kb][1][:, c0: c0 + m]
                                nc.tensor.matmul(
                                    out=ps[:m, :], lhsT=lsl,
                                    rhs=rhsW[layer][kb][:],
                                    start=(kb == 0), stop=(kb == 1))
                            if j % 2 == 1:
                                nc.scalar.copy(
                                    out=stgb[:m, j, : c.D + 2 * c.H],
                                    in_=ps[:m, : c.D + 2 * c.H])
                            else:
                                nc.vector.tensor_copy(
                                    out=stgb[:m, j, : c.D + 2 * c.H],
                                    in_=ps[:m, : c.D + 2 * c.H])
                        ft = rows // P
                        rem = rows - ft * P
                        r0 = lbase + b0 * P
                        if ft:
                            nc.sync.dma_start(
                                out=tab[r0: r0 + ft * P, :WCOL]
                                .rearrange("(t p) f -> p t f", p=P),
                                in_=stg[:, :ft, :])
                        if rem:
                            nc.sync.dma_start(
                                out=tab[r0 + ft * P: r0 + ft * P + rem, :WCOL],
                                in_=stg[:rem, ft, :])

            def build_er_loc():
                start = c.D // 2
                eroff = c.H // 2
                NT1 = (c.NT + 3) // 4
                for half, (tab, idx_sb) in enumerate(
                        [(tabA, erbA_sb), (tabB, erbB_sb)]):
                    for t0, t1 in ((0, NT1), (NT1, 2 * NT1),
                                   (2 * NT1, 3 * NT1), (3 * NT1, c.NT)):
                        nt = t1 - t0
                        eb = erbp.tile([P, NT1, 64], F32, tag="erb")
                        nc.gpsimd.dma_gather(
                            out_ap=eb[:, :nt, :],
                            in_ap=tab[:, start: start + 64],
                            idxs_ap=idx_sb[:, t0 * 8: t1 * 8],
                            num_idxs=nt * P, num_idxs_reg=nt * P,
                            elem_size=64, elem_step=c.ROWF,
                            single_packet=nt * P <= 1024)
                        nc.sync.dma_start(
                            out=er_loc[half * ERR + t0 * P:
                                       half * ERR + t1 * P, 0: c.H // 2]
                            .rearrange("(t p) h -> p t h", p=P),
                            in_=eb[:, :nt, eroff: eroff + c.H // 2])

            def phase_b(layer):
                for b0 in range(0, c.NT, WB):
                    nt_b = min(WB, c.NT - b0)
                    if layer == 0:
                        hstg = hstgp.tile([P, 2, WB, P], BF16, tag="hstg")
                    else:
                        ostg = ostgp.tile([P, WB, c.D], F32, tag="ostg")
                    gtiles = {}
                    for t0 in range(b0, b0 + nt_b):
                        gA2 = gathp.tile([P, 1, KA, c.ROWF], F32, tag="gathA")
                        nc.gpsimd.dma_gather(
                            out_ap=gA2[:, 0, :, :],
                            in_ap=tabA[:, :],
                            idxs_ap=srcA_sb[:, t0 * KA * 8:(t0 + 1) * KA * 8],
                            num_idxs=KA * P, num_idxs_reg=KA * P,
                            elem_size=c.ROWF,
                            single_packet=KA * P <= 1024)
                        gB2 = None
                        if KB:
                            gB2 = gathp.tile([P, 1, KB, c.ROWF], F32,
                                             tag="gathB")
                            nc.gpsimd.dma_gather(
                                out_ap=gB2[:, 0, :, :],
                                in_ap=tabB[:, :],
                                idxs_ap=srcB_sb[:, t0 * KB * 8:(t0 + 1) * KB * 8],
                                num_idxs=KB * P, num_idxs_reg=KB * P,
                                elem_size=c.ROWF,
                                single_packet=KB * P <= 1024)
                        ert2 = ertp.tile([P, 1, KP, 64], F32, tag="ert")
                        nc.gpsimd.dma_gather(
                            out_ap=ert2[:, 0, :, :],
                            in_ap=er_loc[:, :],
                            idxs_ap=dst_sb[:, t0 * KP * 8:(t0 + 1) * KP * 8],
                            num_idxs=KP * P, num_idxs_reg=KP * P,
                            elem_size=64, single_packet=KP * P <= 1024)
                        gtiles[t0] = (gA2, gB2, ert2, 0)
                    for j in range(nt_b):
                        t = b0 + j
                        gA2, gB2, ert2, gi = gtiles[t]
                        gA = gA2[:, gi]
                        gB = gB2[:, gi] if gB2 is not None else None
                        ert = ert2[:, gi] if ert2 is not None else None
                        gbA = gA.bitcast(BF16)
                        gbB = gB.bitcast(BF16) if gB is not None else None
                        ea = smallp.tile([P, K, c.H], F32, tag="eadd")
                        erd_t = ert[:, 0:1, 0: c.H // 2].bitcast(BF16)
                        elA = gbA[:, :, c.D: c.D + c.H]
                        elB = gbB[:, :, c.D: c.D + c.H] if gbB is not None else None
                        nc.vector.tensor_tensor(
                            out=ea[:, 0:KDA, :],
                            in0=elA[:, 0:KDA, :],
                            in1=erd_t.to_broadcast([P, KDA, c.H]),
                            op=mybir.AluOpType.add)
                        nc.vector.tensor_tensor(
                            out=ea[:, KA:KA + KDB, :],
                            in0=elB[:, 0:KDB, :],
                            in1=erd_t.to_broadcast([P, KDB, c.H]),
                            op=mybir.AluOpType.add)
                        if KSA:
                            nc.vector.tensor_tensor(
                                out=ea[:, KDA:KA, :]
                                .rearrange("p (k2 s) h -> p k2 s h", s=2),
                                in0=elA[:, KDA:KA, :]
                                .rearrange("p (k2 s) h -> p k2 s h", s=2),
                                in1=ert[:, 1:1 + KSA // 2, 0: c.H // 2]
                                .bitcast(BF16).unsqueeze(2)
                                .to_broadcast([P, KSA // 2, 2, c.H]),
                                op=mybir.AluOpType.add)
                        if KSB:
                            nc.vector.tensor_tensor(
                                out=ea[:, KA + KDB:K, :]
                                .rearrange("p (k2 s) h -> p k2 s h", s=2),
                                in0=elB[:, KDB:KB, :]
                                .rearrange("p (k2 s) h -> p k2 s h", s=2),
                                in1=ert[:, 1 + KSA // 2:KP, 0: c.H // 2]
                                .bitcast(BF16).unsqueeze(2)
                                .to_broadcast([P, KSB // 2, 2, c.H]),
                                op=mybir.AluOpType.add)
                        tmp = smallp.tile([P, K, c.H], F32, tag="lrtmp")
                        nc.vector.tensor_scalar_mul(tmp[:], ea[:], c.NEG)
                        lr = smallp.tile([P, K, c.H], F32, tag="lrout")
                        nc.vector.tensor_tensor(out=lr[:], in0=ea[:], in1=tmp[:],
                                                op=mybir.AluOpType.max)
                        rm = rhsmp.tile([P, K, DH], BF16, tag="rhsm")
                        nc.scalar.activation(
                            out=rm[:, :, c.D: DH], in_=lr[:],
                            func=mybir.ActivationFunctionType.Exp)
                        nc.vector.tensor_tensor(
                            out=rm[:, 0:KA, : c.D].rearrange(
                                "p k (hd h) -> p k hd h", h=c.H),
                            in0=gbA[:, :, : c.D].rearrange(
                                "p k (hd h) -> p k hd h", h=c.H),
                            in1=rm[:, 0:KA, c.D: DH].unsqueeze(2)
                            .to_broadcast([P, KA, c.HD, c.H]),
                            op=mybir.AluOpType.mult)
                        if KB:
                            nc.vector.tensor_tensor(
                                out=rm[:, KA:K, : c.D].rearrange(
                                    "p k (hd h) -> p k hd h", h=c.H),
                                in0=gbB[:, :, : c.D].rearrange(
                                    "p k (hd h) -> p k hd h", h=c.H),
                                in1=rm[:, KA:K, c.D: DH].unsqueeze(2)
                                .to_broadcast([P, KB, c.HD, c.H]),
                                op=mybir.AluOpType.mult)
                        if KS:
                            s_sc = sallp.tile([P, P, KS], BF16, tag="s_sc")
                            nc.vector.tensor_tensor(
                                out=s_sc[:],
                                in0=iota_rep[:],
                                in1=dstpos_sb[:, t * KS:(t + 1) * KS]
                                .unsqueeze(1).to_broadcast([P, P, KS]),
                                op=mybir.AluOpType.is_equal)
                        ps = psB.tile([P, DH], F32, tag="psB")
                        for ck in range(K):
                            if KDA <= ck < KA:
                                lhsT = s_sc[:, :, ck - KDA]
                            elif ck >= KA + KDB:
                                lhsT = s_sc[:, :, KSA + ck - KA - KDB]
                            else:
                                lhsT = ident[:]
                            nc.tensor.matmul(out=ps[:], lhsT=lhsT,
                                             rhs=rm[:, ck, :],
                                             start=(ck == 0), stop=(ck == K - 1))
                        den = smallp.tile([P, c.H], F32, tag="den")
                        nc.vector.tensor_scalar_max(den[:], ps[:, c.D: DH], 1e-30)
                        rcp = smallp.tile([P, c.H], F32, tag="rcp")
                        nc.vector.reciprocal(rcp[:], den[:])
                        o1 = outp.tile([P, c.D], F32, tag="o1")
                        rcpb = rcp[:].unsqueeze(1).to_broadcast([P, c.HD, c.H])
                        ps4 = ps[:, : c.D].rearrange("p (hd h) -> p hd h", h=c.H)
                        o14 = o1[:].rearrange("p (hd h) -> p hd h", h=c.H)
                        nc.vector.tensor_tensor(out=o14, in0=ps4, in1=rcpb,
                                                op=mybir.AluOpType.mult)
                        nc.vector.tensor_tensor(out=o1[:], in0=o1[:],
                                                in1=b_bc[layer][:],
                                                op=mybir.AluOpType.add)
                        if layer == 0:
                            hb = outp.tile([P, c.D], BF16, tag="hb")
                            nc.scalar.activation(
                                out=hb[:], in_=o1[:],
                                func=mybir.ActivationFunctionType.Relu)
                            for kb in range(2):
                                pst = psT.tile([P, P], F32, tag="psT")
                                nc.tensor.matmul(
                                    out=pst[:],
                                    lhsT=hb[:, kb * P: (kb + 1) * P],
                                    rhs=ident[:], start=True, stop=True)
                                nc.scalar.copy(out=hstg[:, kb, j, :],
                                               in_=pst[:])
                        else:
                            nc.scalar.activation(
                                out=ostg[:, j, :], in_=o1[:],
                                func=mybir.ActivationFunctionType.Relu)
                    if layer == 0:
                        for kb in range(2):
                            nc.sync.dma_start(
                                out=hT_loc[kb * P:(kb + 1) * P,
                                           b0 * P: (b0 + nt_b) * P],
                                in_=hstg[:, kb, :nt_b, :])
                    else:
                        nc.sync.dma_start(
                            out=out_ext[b0 * P: (b0 + nt_b) * P, :]
                            .rearrange("(t p) f -> p t f", p=P),
                            in_=ostg[:, :nt_b, :])

            phase_a(0)
            build_er_loc()
            phase_b(0)
            nc.gpsimd.collective_compute(
                "AllGather", mybir.AluOpType.bypass,
                replica_groups=[list(range(c.cores))],
                ins=[hT_loc[:]], outs=[hT_ag[:]])
            phase_a(1)
            build_er_loc()
            phase_b(1)

    nc.compile()
    return nc



def ref_np(inputs, cfg):
    c = cfg
    x = np.asarray(inputs["data"], np.float64)
    src = np.asarray(inputs["src"]).astype(np.int64)
    dst = np.asarray(inputs["dst"]).astype(np.int64)

    def layer(x, W, al, ar, b):
        N = x.shape[0]
        feat = (x @ np.asarray(W, np.float64)).reshape(N, c.H, c.HD)
        el = np.einsum("nhd,hd->nh", feat, np.asarray(al, np.float64))
        er = np.einsum("nhd,hd->nh", feat, np.asarray(ar, np.float64))
        e = el[src] + er[dst]
        e = np.where(e > 0, e, c.NEG * e)
        m = np.full((N, c.H), -np.inf)
        np.maximum.at(m, dst, e)
        a = np.exp(e - m[dst])
        den = np.zeros((N, c.H))
        np.add.at(den, dst, a)
        alpha = a / den[dst]
        msg = feat[src] * alpha[:, :, None]
        out = np.zeros((N, c.H, c.HD))
        np.add.at(out, dst, msg)
        out = out + np.asarray(b, np.float64).reshape(1, c.H, c.HD)
        return np.maximum(out, 0).reshape(N, c.D)

    h = layer(x, inputs["W1"], inputs["al1"], inputs["ar1"], inputs["b1"])
    h = layer(h, inputs["W2"], inputs["al2"], inputs["ar2"], inputs["b2"])
    return h



_BUILD_CACHE = {}


def kernel(**inputs) -> np.ndarray:
    from concourse.bass_utils import run_bass_kernel_spmd

    cfg = make_cfg(N=50000, E=800000, D=256, H=8, cores=8)
    in_maps, meta = prep_all(inputs, cfg)
    dm = meta.dims
    key = (dm.KA, dm.KB, dm.KS)
    if key not in _BUILD_CACHE:
        _BUILD_CACHE[key] = build_nc(cfg, dm)
    nc = _BUILD_CACHE[key]
    res = run_bass_kernel_spmd(nc, in_maps, list(range(cfg.cores)))
    results = [{"out": res.results[ci]["out"]} for ci in range(cfg.cores)]
    out = finalize(results, cfg, meta)
    return np.ascontiguousarray(out.astype(np.float32))
